# revision 4
# baseline (speedup 1.0000x reference)
"""Trainium2 Bass kernel for nn_Mirror: per-sample conditional flips + fp16 cast.

Full op: x [16,2,64,128,128] f32, x_flag [16], y_flag [16] f32 ->
out [16,2,64,128,128] f16 where per sample b:
  out[b] = 0                 if x_flag[b] <= 0.5
         = flip_h(x[b])      if x_flag[b] > 0.5 and y_flag[b] <= 0.5
         = flip_hw(x[b])     if x_flag[b] > 0.5 and y_flag[b] > 0.5

Sharding: batch 16 -> 8 cores x 2 samples (data parallel, no comms).

Per-core kernel layout trick: a sample [2,64,128,128] is 128 images of
128x128 -> images map to the 128 SBUF partitions, so both flips become
free-dimension reversals (negative-stride SBUF reads) with zero partition
movement.  Runtime flags are handled branch-free: on-chip masks
c1 = (xf>.5)*(1-(yf>.5)), c2 = (xf>.5)*(yf>.5) and
out = revh(x)*c1 + revhw(x)*c2 (ACT does the c2 pass, DVE fuses the rest
via scalar_tensor_tensor).  HBM traffic per core is the minimum:
16 MiB read + 8 MiB write.
"""

import numpy as np

import concourse.bass as bass
import concourse.mybir as mybir
import concourse.tile as tile
from concourse import bacc
from concourse.bass_utils import run_bass_kernel_spmd

N_CORES = 8
FULL_B = 16
B = FULL_B // N_CORES  # samples per core
C, D, W, H = 2, 64, 128, 128
IMG = C * D            # 128 images per sample -> partition dim
WH = W * H             # 16384 free elems per image
CH = 2048              # free-chunk size (16 w-rows)
NCH = WH // CH         # 8 chunks per sample


def build_program():
    nc = bacc.Bacc("TRN2", target_bir_lowering=False, debug=False)
    x = nc.dram_tensor("x", [B, C, D, W, H], mybir.dt.float32, kind="ExternalInput")
    xf = nc.dram_tensor("x_flag", [B], mybir.dt.float32, kind="ExternalInput")
    yf = nc.dram_tensor("y_flag", [B], mybir.dt.float32, kind="ExternalInput")
    out = nc.dram_tensor(
        "out", [B, C, D, W, H], mybir.dt.float16, kind="ExternalOutput"
    )

    xv = x.ap().rearrange("b c d w h -> b (c d) (w h)")  # [B, 128, 16384]
    ov = out.ap().rearrange("b c d w h -> b (c d) (w h)")

    with tile.TileContext(nc) as tc:
        with (
            tc.tile_pool(name="flags", bufs=1) as flag_pool,
            tc.tile_pool(name="in", bufs=6) as in_pool,
            tc.tile_pool(name="mid", bufs=6) as mid_pool,
            tc.tile_pool(name="out", bufs=6) as out_pool,
        ):
            # Replicate flags across all 128 partitions via step-0 DMA read.
            fx = flag_pool.tile([128, B], mybir.dt.float32, tag="fx")
            fy = flag_pool.tile([128, B], mybir.dt.float32, tag="fy")
            nc.sync.dma_start(fx[:], bass.AP(xf, 0, [[0, 128], [1, B]]))
            nc.sync.dma_start(fy[:], bass.AP(yf, 0, [[0, 128], [1, B]]))

            sx = flag_pool.tile([128, B], mybir.dt.float32, tag="sx")
            sy = flag_pool.tile([128, B], mybir.dt.float32, tag="sy")
            c1 = flag_pool.tile([128, B], mybir.dt.float32, tag="c1")
            c2 = flag_pool.tile([128, B], mybir.dt.float32, tag="c2")
            nc.vector.tensor_scalar(sx[:], fx[:], 0.5, None, mybir.AluOpType.is_gt)
            nc.vector.tensor_scalar(sy[:], fy[:], 0.5, None, mybir.AluOpType.is_gt)
            nc.vector.tensor_tensor(c2[:], sx[:], sy[:], mybir.AluOpType.mult)
            nc.vector.tensor_tensor(c1[:], sx[:], c2[:], mybir.AluOpType.subtract)

            for b in range(B):
                c1b = c1[:, b : b + 1]
                c2b = c2[:, b : b + 1]
                for j in range(NCH // 2):
                    k = NCH - 1 - j
                    tj = in_pool.tile([128, CH], mybir.dt.float32, tag="tin")
                    tk = in_pool.tile([128, CH], mybir.dt.float32, tag="tin")
                    nc.sync.dma_start(tj[:], xv[b, :, j * CH : (j + 1) * CH])
                    nc.sync.dma_start(tk[:], xv[b, :, k * CH : (k + 1) * CH])

                    for src_h, src_full, dst in ((tj, tk, j), (tk, tj, k)):
                        # revh: reverse within each 128-elem image row
                        ap_h = src_h[:].rearrange("p (w h) -> p w h", h=H)[:, :, ::-1]
                        # revfull: reverse whole chunk (w-block order and h)
                        ap_full = src_full[:, ::-1]
                        o2 = mid_pool.tile([128, CH], mybir.dt.float32, tag="o2")
                        nc.scalar.mul(o2[:], ap_full, c2b)
                        og = out_pool.tile([128, CH], mybir.dt.float16, tag="og")
                        og3 = og[:].rearrange("p (w h) -> p w h", h=H)
                        o23 = o2[:].rearrange("p (w h) -> p w h", h=H)
                        nc.vector.scalar_tensor_tensor(
                            og3, ap_h, c1b, o23,
                            mybir.AluOpType.mult, mybir.AluOpType.add,
                        )
                        nc.sync.dma_start(
                            ov[b, :, dst * CH : (dst + 1) * CH], og[:]
                        )
    nc.compile()
    return nc


_NC_CACHE = None


def _get_program():
    global _NC_CACHE
    if _NC_CACHE is None:
        _NC_CACHE = build_program()
    return _NC_CACHE


def kernel(x, x_flag, y_flag, _trace=False, **trace_kwargs):
    x = np.ascontiguousarray(np.asarray(x), dtype=np.float32)
    x_flag = np.asarray(x_flag, dtype=np.float32)
    y_flag = np.asarray(y_flag, dtype=np.float32)

    nc = _get_program()
    in_maps = [
        {
            "x": x[i * B : (i + 1) * B],
            "x_flag": x_flag[i * B : (i + 1) * B],
            "y_flag": y_flag[i * B : (i + 1) * B],
        }
        for i in range(N_CORES)
    ]
    res = run_bass_kernel_spmd(
        nc, in_maps, core_ids=list(range(N_CORES)), trace=_trace, **trace_kwargs
    )
    out = np.concatenate([res.results[i]["out"] for i in range(N_CORES)], axis=0)
    if _trace:
        return out, res
    return out


# revision 9
# speedup vs baseline: 1.2396x; 1.2396x over previous
"""Trainium2 Bass kernel for nn_Mirror: per-sample conditional flips + fp16 cast.

Full op: x [16,2,64,128,128] f32, x_flag [16], y_flag [16] f32 ->
out [16,2,64,128,128] f16 where per sample b:
  out[b] = 0                 if x_flag[b] <= 0.5
         = flip_h(x[b])      if x_flag[b] > 0.5 and y_flag[b] <= 0.5
         = flip_hw(x[b])     if x_flag[b] > 0.5 and y_flag[b] > 0.5

Sharding: batch 16 -> 8 cores x 2 samples (data parallel, no comms).

Per-core kernel layout trick: a sample [2,64,128,128] is 128 images of
128x128 -> images map to the 128 SBUF partitions, so both flips become
free-dimension reversals (negative-stride SBUF reads) with zero partition
movement.  Runtime flags are handled branch-free: on-chip masks
c1 = (xf>.5)*(1-(yf>.5)), c2 = (xf>.5)*(yf>.5) and
out = revh(x)*c1 + revhw(x)*c2 (ACT does the c2 pass, DVE fuses the rest
via scalar_tensor_tensor).  HBM traffic per core is the minimum:
16 MiB read + 8 MiB write.
"""

import numpy as np

import concourse.bass as bass
import concourse.mybir as mybir
import concourse.tile as tile
from concourse import bacc
from concourse.bass_utils import run_bass_kernel_spmd

N_CORES = 8
FULL_B = 16
B = FULL_B // N_CORES  # samples per core
C, D, W, H = 2, 64, 128, 128
IMG = C * D            # 128 images per sample -> partition dim
WH = W * H             # 16384 free elems per image
CH = 2048              # free-chunk size (16 w-rows)
NCH = WH // CH         # 8 chunks per sample


def build_program():
    nc = bacc.Bacc("TRN2", target_bir_lowering=False, debug=False)
    x = nc.dram_tensor("x", [B, C, D, W, H], mybir.dt.float32, kind="ExternalInput")
    xf = nc.dram_tensor("x_flag", [B], mybir.dt.float32, kind="ExternalInput")
    yf = nc.dram_tensor("y_flag", [B], mybir.dt.float32, kind="ExternalInput")
    out = nc.dram_tensor(
        "out", [B, C, D, W, H], mybir.dt.float16, kind="ExternalOutput"
    )

    xv = x.ap().rearrange("b c d w h -> b (c d) (w h)")  # [B, 128, 16384]
    ov = out.ap().rearrange("b c d w h -> b (c d) (w h)")

    with tile.TileContext(nc) as tc:
        with (
            tc.tile_pool(name="flags", bufs=1) as flag_pool,
            tc.tile_pool(name="in", bufs=8) as in_pool,
            tc.tile_pool(name="mid", bufs=6) as mid_pool,
            tc.tile_pool(name="out", bufs=6) as out_pool,
        ):
            # Replicate flags across all 128 partitions via step-0 DMA read.
            fx = flag_pool.tile([128, B], mybir.dt.float32, tag="fx")
            fy = flag_pool.tile([128, B], mybir.dt.float32, tag="fy")
            nc.sync.dma_start(fx[:], bass.AP(xf, 0, [[0, 128], [1, B]]))
            nc.sync.dma_start(fy[:], bass.AP(yf, 0, [[0, 128], [1, B]]))

            sx = flag_pool.tile([128, B], mybir.dt.float32, tag="sx")
            sy = flag_pool.tile([128, B], mybir.dt.float32, tag="sy")
            c1 = flag_pool.tile([128, B], mybir.dt.float32, tag="c1")
            c2 = flag_pool.tile([128, B], mybir.dt.float32, tag="c2")
            nc.vector.tensor_scalar(sx[:], fx[:], 0.5, None, mybir.AluOpType.is_gt)
            nc.vector.tensor_scalar(sy[:], fy[:], 0.5, None, mybir.AluOpType.is_gt)
            nc.vector.tensor_tensor(c2[:], sx[:], sy[:], mybir.AluOpType.mult)
            nc.vector.tensor_tensor(c1[:], sx[:], c2[:], mybir.AluOpType.subtract)

            n_store = 0
            for b in range(B):
                c1b = c1[:, b : b + 1]
                c2b = c2[:, b : b + 1]
                for j in range(NCH // 2):
                    k = NCH - 1 - j
                    tj = in_pool.tile([128, CH], mybir.dt.float32, tag="tin")
                    tk = in_pool.tile([128, CH], mybir.dt.float32, tag="tin")
                    nc.sync.dma_start(tj[:], xv[b, :, j * CH : (j + 1) * CH])
                    nc.sync.dma_start(tk[:], xv[b, :, k * CH : (k + 1) * CH])

                    for src_h, src_full, dst in ((tj, tk, j), (tk, tj, k)):
                        # revh: reverse within each 128-elem image row
                        ap_h = src_h[:].rearrange("p (w h) -> p w h", h=H)[:, :, ::-1]
                        # revfull: reverse whole chunk (w-block order and h)
                        ap_full = src_full[:, ::-1]
                        o2 = mid_pool.tile([128, CH], mybir.dt.float32, tag="o2")
                        nc.scalar.mul(o2[:], ap_full, c2b)
                        og = out_pool.tile([128, CH], mybir.dt.float16, tag="og")
                        og3 = og[:].rearrange("p (w h) -> p w h", h=H)
                        o23 = o2[:].rearrange("p (w h) -> p w h", h=H)
                        nc.vector.scalar_tensor_tensor(
                            og3, ap_h, c1b, o23,
                            mybir.AluOpType.mult, mybir.AluOpType.add,
                        )
                        # stores on gpsimd SWDGE: store waits on DVE must not
                        # head-of-line-block the load stream on sync's queue
                        nc.gpsimd.dma_start(
                            ov[b, :, dst * CH : (dst + 1) * CH], og[:]
                        )
                        n_store += 1
    nc.compile()
    return nc


_NC_CACHE = None


def _get_program():
    global _NC_CACHE
    if _NC_CACHE is None:
        _NC_CACHE = build_program()
    return _NC_CACHE


def kernel(x, x_flag, y_flag, _trace=False, **trace_kwargs):
    x = np.ascontiguousarray(np.asarray(x), dtype=np.float32)
    x_flag = np.asarray(x_flag, dtype=np.float32)
    y_flag = np.asarray(y_flag, dtype=np.float32)

    nc = _get_program()
    in_maps = [
        {
            "x": x[i * B : (i + 1) * B],
            "x_flag": x_flag[i * B : (i + 1) * B],
            "y_flag": y_flag[i * B : (i + 1) * B],
        }
        for i in range(N_CORES)
    ]
    res = run_bass_kernel_spmd(
        nc, in_maps, core_ids=list(range(N_CORES)), trace=_trace, **trace_kwargs
    )
    out = np.concatenate([res.results[i]["out"] for i in range(N_CORES)], axis=0)
    if _trace:
        return out, res
    return out


# revision 10
# speedup vs baseline: 1.3803x; 1.1136x over previous
"""Trainium2 Bass kernel for nn_Mirror: per-sample conditional flips + fp16 cast.

Full op: x [16,2,64,128,128] f32, x_flag [16], y_flag [16] f32 ->
out [16,2,64,128,128] f16 where per sample b:
  out[b] = 0                 if x_flag[b] <= 0.5
         = flip_h(x[b])      if x_flag[b] > 0.5 and y_flag[b] <= 0.5
         = flip_hw(x[b])     if x_flag[b] > 0.5 and y_flag[b] > 0.5

Sharding: batch 16 -> 8 cores x 2 samples (data parallel, no comms).

Per-core kernel layout trick: a sample [2,64,128,128] is 128 images of
128x128 -> images map to the 128 SBUF partitions, so both flips become
free-dimension reversals (negative-stride SBUF reads) with zero partition
movement.  Runtime flags are handled branch-free: on-chip masks
c1 = (xf>.5)*(1-(yf>.5)), c2 = (xf>.5)*(yf>.5) and
out = revh(x)*c1 + revhw(x)*c2 (ACT does the c2 pass, DVE fuses the rest
via scalar_tensor_tensor).  HBM traffic per core is the minimum:
16 MiB read + 8 MiB write.
"""

import numpy as np

import concourse.bass as bass
import concourse.mybir as mybir
import concourse.tile as tile
from concourse import bacc
from concourse.bass_utils import run_bass_kernel_spmd

N_CORES = 8
FULL_B = 16
B = FULL_B // N_CORES  # samples per core
C, D, W, H = 2, 64, 128, 128
IMG = C * D            # 128 images per sample -> partition dim
WH = W * H             # 16384 free elems per image
CH = 2048              # free-chunk size (16 w-rows)
NCH = WH // CH         # 8 chunks per sample


def build_program():
    nc = bacc.Bacc("TRN2", target_bir_lowering=False, debug=False)
    x = nc.dram_tensor("x", [B, C, D, W, H], mybir.dt.float32, kind="ExternalInput")
    xf = nc.dram_tensor("x_flag", [B], mybir.dt.float32, kind="ExternalInput")
    yf = nc.dram_tensor("y_flag", [B], mybir.dt.float32, kind="ExternalInput")
    out = nc.dram_tensor(
        "out", [B, C, D, W, H], mybir.dt.float16, kind="ExternalOutput"
    )

    xv = x.ap().rearrange("b c d w h -> b (c d) (w h)")  # [B, 128, 16384]
    ov = out.ap().rearrange("b c d w h -> b (c d) (w h)")

    with tile.TileContext(nc) as tc:
        with (
            tc.tile_pool(name="flags", bufs=1) as flag_pool,
            tc.tile_pool(name="in", bufs=10) as in_pool,
            tc.tile_pool(name="mid", bufs=7) as mid_pool,
            tc.tile_pool(name="out", bufs=7) as out_pool,
        ):
            # Replicate flags across all 128 partitions via step-0 DMA read.
            fx = flag_pool.tile([128, B], mybir.dt.float32, tag="fx")
            fy = flag_pool.tile([128, B], mybir.dt.float32, tag="fy")
            nc.sync.dma_start(fx[:], bass.AP(xf, 0, [[0, 128], [1, B]]))
            nc.sync.dma_start(fy[:], bass.AP(yf, 0, [[0, 128], [1, B]]))

            sx = flag_pool.tile([128, B], mybir.dt.float32, tag="sx")
            sy = flag_pool.tile([128, B], mybir.dt.float32, tag="sy")
            c1 = flag_pool.tile([128, B], mybir.dt.float32, tag="c1")
            c2 = flag_pool.tile([128, B], mybir.dt.float32, tag="c2")
            nc.vector.tensor_scalar(sx[:], fx[:], 0.5, None, mybir.AluOpType.is_gt)
            nc.vector.tensor_scalar(sy[:], fy[:], 0.5, None, mybir.AluOpType.is_gt)
            nc.vector.tensor_tensor(c2[:], sx[:], sy[:], mybir.AluOpType.mult)
            nc.vector.tensor_tensor(c1[:], sx[:], c2[:], mybir.AluOpType.subtract)

            n_store = 0
            for b in range(B):
                c1b = c1[:, b : b + 1]
                c2b = c2[:, b : b + 1]
                for j in range(NCH // 2):
                    k = NCH - 1 - j
                    tj = in_pool.tile([128, CH], mybir.dt.float32, tag="tin")
                    tk = in_pool.tile([128, CH], mybir.dt.float32, tag="tin")
                    nc.sync.dma_start(tj[:], xv[b, :, j * CH : (j + 1) * CH])
                    nc.sync.dma_start(tk[:], xv[b, :, k * CH : (k + 1) * CH])

                    for src_h, src_full, dst in ((tj, tk, j), (tk, tj, k)):
                        # revh: reverse within each 128-elem image row
                        ap_h = src_h[:].rearrange("p (w h) -> p w h", h=H)[:, :, ::-1]
                        # revfull: reverse whole chunk (w-block order and h)
                        ap_full = src_full[:, ::-1]
                        o2 = mid_pool.tile([128, CH], mybir.dt.float32, tag="o2")
                        nc.scalar.mul(o2[:], ap_full, c2b)
                        og = out_pool.tile([128, CH], mybir.dt.float16, tag="og")
                        og3 = og[:].rearrange("p (w h) -> p w h", h=H)
                        o23 = o2[:].rearrange("p (w h) -> p w h", h=H)
                        nc.vector.scalar_tensor_tensor(
                            og3, ap_h, c1b, o23,
                            mybir.AluOpType.mult, mybir.AluOpType.add,
                        )
                        # stores on gpsimd SWDGE: store waits on DVE must not
                        # head-of-line-block the load stream on sync's queue
                        nc.gpsimd.dma_start(
                            ov[b, :, dst * CH : (dst + 1) * CH], og[:]
                        )
                        n_store += 1
    nc.compile()
    return nc


_NC_CACHE = None


def _get_program():
    global _NC_CACHE
    if _NC_CACHE is None:
        _NC_CACHE = build_program()
    return _NC_CACHE


def kernel(x, x_flag, y_flag, _trace=False, **trace_kwargs):
    x = np.ascontiguousarray(np.asarray(x), dtype=np.float32)
    x_flag = np.asarray(x_flag, dtype=np.float32)
    y_flag = np.asarray(y_flag, dtype=np.float32)

    nc = _get_program()
    in_maps = [
        {
            "x": x[i * B : (i + 1) * B],
            "x_flag": x_flag[i * B : (i + 1) * B],
            "y_flag": y_flag[i * B : (i + 1) * B],
        }
        for i in range(N_CORES)
    ]
    res = run_bass_kernel_spmd(
        nc, in_maps, core_ids=list(range(N_CORES)), trace=_trace, **trace_kwargs
    )
    out = np.concatenate([res.results[i]["out"] for i in range(N_CORES)], axis=0)
    if _trace:
        return out, res
    return out


# revision 11
# speedup vs baseline: 1.6833x; 1.2195x over previous
"""Trainium2 Bass kernel for nn_Mirror: per-sample conditional flips + fp16 cast.

Full op: x [16,2,64,128,128] f32, x_flag [16], y_flag [16] f32 ->
out [16,2,64,128,128] f16 where per sample b:
  out[b] = 0                 if x_flag[b] <= 0.5
         = flip_h(x[b])      if x_flag[b] > 0.5 and y_flag[b] <= 0.5
         = flip_hw(x[b])     if x_flag[b] > 0.5 and y_flag[b] > 0.5

Device kernel (per core, 2 sample slots):
  A sample [2,64,128,128] is 128 images of 128x128 -> images map to the 128
  SBUF partitions, so both flips are free-dim manipulations.  Per 2048-elem
  free chunk j of a sample:
    load  T <- x[b] chunk j            (sync HWDGE, cond = x_flag[b] > 0.5)
    O = revh(T) cast fp16              (single 1-input pass, DVE/ACT alternate)
    store out[b] chunk j     <- O      (gpsimd SWDGE, cond = active & !yflip)
    store out[b] chunk 7-j   <- revw(O)(gpsimd SWDGE, cond = active &  yflip;
                                        w-reversal on the SBUF-side read AP)
  Flag compares run on raw float bits in engine registers (signed-int compare
  against bits(0.5f) matches float > 0.5 for non-NaN inputs).  Skipped DMAs
  still bump their semaphores, so Tile's schedule is oblivious to the flags.
  Inactive samples move zero bytes; output DRAM is pre-zeroed by the runtime.

Host scheduling: the flags are host-visible, so active samples are assigned
round-robin to (core, slot) across the 8 cores; inactive samples never ship
and their output stays host-side zeros.  With k active samples the busiest
core processes ceil(k/8) samples, i.e. half traffic whenever k <= 8.
"""

import numpy as np

import concourse.bass as bass
import concourse.mybir as mybir
import concourse.tile as tile
from concourse import bacc
from concourse.bass_utils import run_bass_kernel_spmd
from concourse.expressions import s_logical_and
from concourse.ordered_set import OrderedSet

N_CORES = 8
FULL_B = 16
B = 2                  # sample slots per core
C, D, W, H = 2, 64, 128, 128
WH = W * H             # 16384 free elems per image
CH = 2048              # free-chunk size (16 w-rows)
NCH = WH // CH         # 8 chunks per sample
F_HALF = 0x3F000000    # bits of 0.5f

SP = mybir.EngineType.SP
POOL = mybir.EngineType.Pool


def build_program(sim_init=False):
    nc = bacc.Bacc("TRN2", target_bir_lowering=False, debug=False)
    x = nc.dram_tensor("x", [B, C, D, W, H], mybir.dt.float32, kind="ExternalInput")
    xf = nc.dram_tensor("x_flag", [B], mybir.dt.float32, kind="ExternalInput")
    yf = nc.dram_tensor("y_flag", [B], mybir.dt.float32, kind="ExternalInput")
    out = nc.dram_tensor(
        "out", [B, C, D, W, H], mybir.dt.float16, kind="ExternalOutput"
    )

    xv = x.ap().rearrange("b c d w h -> b (c d) (w h)")  # [B, 128, 16384]
    ov = out.ap().rearrange("b c d w h -> b (c d) (w h)")

    with tile.TileContext(nc) as tc:
        with (
            tc.tile_pool(name="flags", bufs=1) as flag_pool,
            tc.tile_pool(name="in", bufs=12) as in_pool,
            tc.tile_pool(name="out", bufs=10) as out_pool,
        ):
            fx = flag_pool.tile([1, B], mybir.dt.float32, tag="fx")
            fy = flag_pool.tile([1, B], mybir.dt.float32, tag="fy")
            nc.sync.dma_start(fx[:], xf.ap().unsqueeze(0))
            nc.sync.dma_start(fy[:], yf.ap().unsqueeze(0))

            # per-sample flag bits in registers on the DMA-issuing engines
            conds = []
            for b in range(B):
                xr = nc.alloc_registers(f"xr{b}", engines=OrderedSet([SP, POOL]))
                yr = nc.alloc_registers(f"yr{b}", engines=OrderedSet([POOL]))
                nc.regs_load(xr, fx[0:1, b : b + 1].bitcast(mybir.dt.int32))
                nc.regs_load(yr, fy[0:1, b : b + 1].bitcast(mybir.dt.int32))
                act_sp = nc.snap(xr, engines=OrderedSet([SP])) > F_HALF
                xg = nc.snap(xr, engines=OrderedSet([POOL])) > F_HALF
                yg = nc.snap(yr, engines=OrderedSet([POOL])) > F_HALF
                yl = nc.snap(yr, engines=OrderedSet([POOL])) <= F_HALF
                conds.append(
                    (act_sp, s_logical_and(xg, yl), s_logical_and(xg, yg))
                )

            n = 0
            for b in range(B):
                act_sp, c_a, c_b = conds[b]
                for j in range(NCH):
                    jb = NCH - 1 - j
                    t = in_pool.tile([128, CH], mybir.dt.float32, tag="tin")
                    if sim_init:
                        # CoreSim-only: skipped loads leave tiles uninit,
                        # which the sim rejects; HW reads garbage that is
                        # never stored.
                        nc.gpsimd.memset(t[:], 0.0)
                    nc.sync.dma_start(
                        t[:], xv[b, :, j * CH : (j + 1) * CH], cond=act_sp
                    )
                    og = out_pool.tile([128, CH], mybir.dt.float16, tag="og")
                    # O = revh(T), cast to fp16; single 1-input pass
                    src = t[:].rearrange("p (w h) -> p w h", h=H)[:, :, ::-1]
                    dst = og[:].rearrange("p (w h) -> p w h", h=H)
                    if n % 2 == 0:
                        nc.vector.tensor_copy(dst, src)
                    else:
                        nc.scalar.copy(dst, src)
                    n += 1
                    # store A: no w-flip
                    nc.gpsimd.dma_start(
                        ov[b, :, j * CH : (j + 1) * CH], og[:], cond=c_a
                    )
                    # store B: w-flip via reversed w-block read of O
                    og_wrev = og[:].rearrange("p (w h) -> p w h", h=H)[:, ::-1, :]
                    nc.gpsimd.dma_start(
                        ov[b, :, jb * CH : (jb + 1) * CH], og_wrev, cond=c_b
                    )
    nc.compile()
    return nc


_NC_CACHE = None


def _get_program():
    global _NC_CACHE
    if _NC_CACHE is None:
        _NC_CACHE = build_program()
    return _NC_CACHE


def kernel(x, x_flag, y_flag, _trace=False, **trace_kwargs):
    x = np.asarray(x)
    if x.dtype != np.float32:
        x = x.astype(np.float32)
    x_flag = np.asarray(x_flag, dtype=np.float32)
    y_flag = np.asarray(y_flag, dtype=np.float32)
    n = x.shape[0]
    assert n == FULL_B, x.shape

    # host-side schedule: only active samples do device work; spread them
    # round-robin over cores so the busiest core gets ceil(k / n_cores)
    active = [int(i) for i in np.nonzero(x_flag > 0.5)[0]]
    # (core, slot) -> sample index
    assign = {}
    for i, idx in enumerate(active):
        assign[(i % N_CORES, i // N_CORES)] = idx
    assert len(active) <= N_CORES * B

    sample_shape = x.shape[1:]
    dummy = np.zeros((1,) + sample_shape, dtype=np.float32)
    in_maps = []
    for c in range(N_CORES):
        xs, xfs, yfs = [], [], []
        for s in range(B):
            idx = assign.get((c, s))
            if idx is None:
                xs.append(dummy[0])
                xfs.append(0.0)
                yfs.append(0.0)
            else:
                xs.append(x[idx])
                xfs.append(float(x_flag[idx]))
                yfs.append(float(y_flag[idx]))
        in_maps.append(
            {
                "x": np.stack(xs),
                "x_flag": np.array(xfs, dtype=np.float32),
                "y_flag": np.array(yfs, dtype=np.float32),
            }
        )

    nc = _get_program()
    res = run_bass_kernel_spmd(
        nc, in_maps, core_ids=list(range(N_CORES)), trace=_trace, **trace_kwargs
    )

    out = np.zeros((FULL_B,) + sample_shape, dtype=np.float16)
    for (c, s), idx in assign.items():
        out[idx] = res.results[c]["out"][s]
    if _trace:
        return out, res
    return out
